# revision 92
# baseline (speedup 1.0000x reference)
"""Trainium2 Bass kernel for nn_LinearTemporalSelfAttention (B=4,T=8192,D=512,H=8).

Sharding: 8 cores = B(4) x T-halves(2). Each core owns a (b, t-half) slab
(4096 x 512) end-to-end; cross-core data is only the KV-state einsum
(sum over full T), AllReduced pair-wise.

v2 design (trace-driven rewrite of the v1 kernel):
 - Host computes LN1 ((x-mu)*rstd, exact f32; gamma/beta folded into the
   QKV weights/biases as before) and ships xn TRANSPOSED per core as
   bf16 [D, TH]. The residual x + h and the tiny emb/stylization-vector
   path (silu(emb)@emb_W) also run on host. Device input traffic halves.
 - ZERO on-device transposes (v1 spent 474us on 384 DMA_TRANSPOSEs):
   q is computed transposed (lhsT=Wq chunks stationary, rhs=xnT moving)
   and k/v in normal layout (lhsT=xnT chunks stationary, rhs=Wk/Wv) --
   both straight off the same xnT tiles. Phase B stays fully transposed
   (y.T = attn2.T @ qeT; out-proj consumes hs.T directly) and the kernel
   emits h.T; the host transposes/adds the residual.
 - No GpSimd elementwise ops (v1: 360us of Q7 software overhead), and no
   big DVE reciprocals (v1: 113us of 8cy/elem iterative divides):
   1/qsum is exp(-ln(qsum)) batched over [8, TH] on ACT; silu is
   0.5*x*(1+tanh(x/2)) with the 0.5 folded into out_W on host.
 - ACT table loads: v1 ping-ponged ln<->exp sets 125x (160us). All Ln
   usage is batched at two points (1/qsum prologue, LN2 rstd between
   B1/B2); everything else uses exp/tanh/square/copy from one set.
   ~5 loads total.
 - Per-token scalars in transposed layout (1/qsum rows, LN2 m2/rstd2,
   stylization scale/shift) are applied via tiny PE rank-1/broadcast
   matmuls into PSUM + fused DVE tensor-tensor passes.
"""
import numpy as np
import ml_dtypes

B, T, D, H, TE = 4, 8192, 512, 8, 2048
Dh = D // H          # 64
EPS = 1e-5
NCORES = 8
TH = T // 2          # 4096 rows per core
P = 128
KC = D // P          # 4 chunks of the feature dim
TS = 512             # t-columns per phase chunk
TC = TH // TS        # 8 t-chunks per core
NSUB = TS // P       # 4 row-subtiles per t-chunk
NT = TH // P         # 32 row subtiles total
CCU = 64 * H * (Dh + 1)     # 33280 floats of U_aug

_CACHE: dict = {}


def _build(flags):
    has_bq, has_bk, has_bv = flags
    from contextlib import ExitStack
    import concourse.bass as bass
    import concourse.bacc as bacc
    import concourse.tile as tile
    import concourse.mybir as mybir

    f32 = mybir.dt.float32
    bf16 = mybir.dt.bfloat16
    f8 = mybir.dt.float8e4
    DR = mybir.MatmulPerfMode.DoubleRow
    Alu = mybir.AluOpType
    Act = mybir.ActivationFunctionType

    nc = bacc.Bacc("TRN2", target_bir_lowering=False, debug=False,
                   enable_asserts=True, num_devices=NCORES)

    # host ships partition-major layouts so every DMA lands as >=2KB
    # contiguous runs per partition (512B strided runs cap at ~40% BW)
    xn_in = nc.declare_dram_parameter("xn", [P, TC, KC, TS], f8, isOutput=False)
    mk_in = nc.declare_dram_parameter("mask", [TH], f32, isOutput=False)
    wq_in = nc.declare_dram_parameter("wq", [P, KC, D], f8, isOutput=False)
    wk_in = nc.declare_dram_parameter("wk", [P, KC, D], f8, isOutput=False)
    wv_in = nc.declare_dram_parameter("wv", [P, KC, D], f8, isOutput=False)
    wo_in = nc.declare_dram_parameter("wo", [P, KC, D], bf16, isOutput=False)
    vec_in = nc.declare_dram_parameter("vecs", [1, 5, D], f32, isOutput=False)
    hp_in = nc.declare_dram_parameter("hpair", [9, KC, P], bf16, isOutput=False)
    h_out = nc.declare_dram_parameter("y", [KC, P, TH], bf16, isOutput=True)

    PAIRS = [[0, 1], [2, 3], [4, 5], [6, 7]]

    with tile.TileContext(nc) as tc, ExitStack() as ctx:
        const = ctx.enter_context(tc.tile_pool(name="const", bufs=1))
        wpool = ctx.enter_context(tc.tile_pool(name="wpool", bufs=1))
        qstash = ctx.enter_context(tc.tile_pool(name="qstash", bufs=1))
        dramp = ctx.enter_context(tc.tile_pool(name="dram", bufs=1, space="DRAM"))

        eps_t = const.tile([P, 1], f32)
        nc.vector.memset(eps_t, EPS)
        ones_row = const.tile([1, P], bf16)
        nc.vector.memset(ones_row, 1.0)
        ones_col = const.tile([P, 1], bf16)
        nc.vector.memset(ones_col, 1.0)
        # pairones8[p, c, m] = 1 if head m = 2c + (p>=64): per-chunk qsum
        # reduction lhsT with M=16 output (cols 8:16 stay zero; rows of
        # other chunks stay 0 so the PSUM accumulates all four chunks).
        # fp8 + 16-wide so the qsum / y2sum reductions run in DoubleRow
        # mode (dual-fp8 ldweights needs a 16B outer stride).
        pairones8 = const.tile([P, KC, 16], f8)
        nc.vector.memset(pairones8, 0.0)
        for c in range(KC):
            nc.vector.memset(pairones8[0:64, c, 2 * c:2 * c + 1], 1.0)
            nc.vector.memset(pairones8[64:P, c, 2 * c + 1:2 * c + 2], 1.0)
        # bf16 twin for the (bf16) y^2 sums: y^2 spans ~1e-6..10 across
        # tokens, far outside e4m3's subnormal floor, so the y2 path
        # cannot run in fp8 (underflow -> negative var -> NaN)
        pairones8b = const.tile([P, KC, 8], bf16)
        nc.vector.memset(pairones8b, 0.0)
        for c in range(KC):
            nc.vector.memset(pairones8b[0:64, c, 2 * c:2 * c + 1], 1.0)
            nc.vector.memset(pairones8b[64:P, c, 2 * c + 1:2 * c + 2], 1.0)
        # ones8x8: all-ones [8,8] -> sum-over-heads with 8x replication
        ones8x8 = const.tile([8, 8], bf16)
        nc.vector.memset(ones8x8, 1.0)
        # caff[m, c, p] = head indicator for m<8; caff[8, c, p] = 64*C/A at
        # feature c*128+p: lhsT of the K=9 affine matmul producing
        # 64*(nm2[t] + (C/A)[l]). Indicator rows first so all writable
        # rows sit at partition base 0 (HW requires 0/32/64/96 bases).
        caff_s = const.tile([9, KC, P], bf16)

        # startup-critical DMAs first: sweep 1 only needs wk/wv/mask (and
        # the first xn chunk, issued right below). wq (sweep 2), wo/vecs/
        # ahp (phase B) land during sweep 1.
        # q/k/v weights are fp8, pre-scaled by 64 on host (W std ~0.02 would
        # drown in e4m3 subnormals); the 1/64 is folded into the Exp ACT
        # scale (k, q) and the mask64 column scale (v)
        wq_s = wpool.tile([P, KC, D], f8)
        wk_s = wpool.tile([P, KC, D], f8)
        wv_s = wpool.tile([P, KC, D], f8)
        wo_s = wpool.tile([P, KC, D], bf16)
        # lnmask: 0 where kept, -30 where masked; folded into the k-path
        # Exp bias so et = exp(k)*mask needs no separate va mask multiply
        lnm_s = wpool.tile([P, NT], f32)
        vec_s = wpool.tile([1, 5, D], f32)
        # startup-critical loads in parallel across the three DMA-capable
        # queues (all rings come up ~9us into the launch preamble)
        nc.scalar.dma_start(out=wk_s, in_=wk_in[:])
        nc.gpsimd.dma_start(out=wv_s, in_=wv_in[:])

        qe_s = qstash.tile([P, KC, TH], f8)       # exp(q) transposed
        qsum_sb = qstash.tile([8, TH], f32)       # per-head q softmax sums
        rq_bf = qstash.tile([8, TH], bf16)        # 1/qsum
        rq64_bf = qstash.tile([8, TH], bf16)      # 64/qsum (fp8-scale mgmt)
        # attn2/saug allocated up front: their zero-fills run during the
        # startup DMA wait instead of on the post-collective critical path
        attn2 = qstash.tile([P, KC, P], bf16)
        nc.vector.memset(attn2, 0.0)
        saug = qstash.tile([P, KC, 16], f8)
        nc.vector.memset(saug, 0.0)

        # bf16 collective payload: halves the AllReduce bytes; U entries
        # are ~8192-token sums so the 0.4% rounding washes into ~0.5% of
        # attn2 (acceptable within the 2e-2 gate)
        cc_in_t = [dramp.tile([CCU], bf16, tag=f"cci{h}", name=f"cci{h}")
                   for h in range(2)]
        cc_out_t = [dramp.tile([CCU], bf16, tag=f"cco{h}", name=f"cco{h}")
                    for h in range(2)]

        # ================= phase A =================
        with ExitStack() as ctxA:
            xpool = ctxA.enter_context(tc.tile_pool(name="xpool", bufs=1))
            work = ctxA.enter_context(tc.tile_pool(name="work", bufs=4))
            # sweep-1 PSUM pools live only until the U copies ship; sweep 2
            # then opens its own pools (sweep-1's single-buffered pk/pv was
            # the dominant pacing bug: every pv matmul stalled ~1.3us on
            # the previous subtile's va DVE reads)
            ctxS1 = ExitStack()
            psK = ctxS1.enter_context(tc.tile_pool(name="psK", bufs=3, space="PSUM"))
            psV = ctxS1.enter_context(tc.tile_pool(name="psV", bufs=3, space="PSUM"))
            psU = ctxS1.enter_context(tc.tile_pool(name="psU", bufs=1, space="PSUM"))

            xn_s = xpool.tile([P, TC, KC, TS], f8)
            # first xn chunk next (startup critical), then the DMAs sweep 1
            # doesn't need
            nc.sync.dma_start(out=xn_s[:, 0], in_=xn_in[:, 0])
            nc.sync.dma_start(out=lnm_s,
                              in_=mk_in[:].rearrange("(n p) -> p n", p=P))
            nc.sync.dma_start(out=wq_s, in_=wq_in[:])
            nc.sync.dma_start(out=caff_s, in_=hp_in[:])
            nc.sync.dma_start(out=vec_s, in_=vec_in[:])
            nc.sync.dma_start(out=wo_s, in_=wo_in[:])

            bk_row = None
            if has_bk:
                bk_row = const.tile([1, D], bf16)
                nc.vector.tensor_copy(out=bk_row, in_=vec_s[:, 4, :])
            bv_row = None
            if has_bv:
                bv_row = const.tile([1, D], bf16)
                nc.vector.tensor_copy(out=bv_row, in_=vec_s[:, 3, :])

            # head-pair-packed U: pair p occupies [128, p%2, 130] of u0/u1;
            # quadrants [0:64, 0:65] and [64:128, 65:130] hold the two
            # heads' U_aug, the other two quadrants are ignored cross-terms
            u0 = psU.tile([P, 2, 2 * (Dh + 1)], f32, tag="u0")
            u1 = psU.tile([P, 2, 2 * (Dh + 1)], f32, tag="u1")

            # ---- sweep 1: k/v + U accumulation (feeds the AllReduce
            # as early as possible; the q sweep then runs DURING the
            # collective so the PE never idles through it). U matmuls
            # for a pair are issued one pair LATE so they never wait on
            # the et/va chain (any sub-us PE stall resets the 3us DVFS
            # ramp and halves the clock) ----
            u_pend = None
            u_cnt = 0

            def emit_u(pend, start, stop):
                pet2, pva2 = pend
                for p in range(4):
                    u = u0 if p < 2 else u1
                    nc.tensor.matmul(
                        out=u[:, p % 2, :],
                        lhsT=pet2[:, :, p * P:(p + 1) * P],
                        rhs=pva2[:, :, p, :],
                        perf_mode=DR,
                        start=(start and p % 2 == 0),
                        stop=(stop and p % 2 == 1))

            for ci in range(TC):
                if ci > 0:
                    nc.sync.dma_start(out=xn_s[:, ci], in_=xn_in[:, ci])
                for ti in range(NSUB):
                    i = ci * NSUB + ti
                    ssl = slice(ti * P, (ti + 1) * P)
                    pk = psK.tile([P, D], f32, tag="pk")
                    pv = psV.tile([P, D], f32, tag="pv")
                    for j in range(0, KC, 2):
                        nc.tensor.matmul(out=pk,
                                         lhsT=xn_s[:, ci, j:j + 2, ssl],
                                         rhs=wk_s[:, j:j + 2, :],
                                         perf_mode=DR,
                                         start=(j == 0),
                                         stop=(j == KC - 2 and not has_bk))
                        nc.tensor.matmul(out=pv,
                                         lhsT=xn_s[:, ci, j:j + 2, ssl],
                                         rhs=wv_s[:, j:j + 2, :],
                                         perf_mode=DR,
                                         start=(j == 0),
                                         stop=(j == KC - 2 and not has_bv))
                    if has_bk:
                        nc.tensor.matmul(out=pk, lhsT=ones_row, rhs=bk_row,
                                         start=False, stop=True)
                    if has_bv:
                        nc.tensor.matmul(out=pv, lhsT=ones_row, rhs=bv_row,
                                         start=False, stop=True)
                    # et = exp(k)*mask via the ln(mask) ACT bias: the va
                    # mask multiplies disappear (ones-cols memset once per
                    # pair). et/va fp8, paired across two consecutive
                    # subtiles: U accumulation runs in DoubleRow (U
                    # averages over 8192 tokens, e4m3 noise washes out).
                    sub = i % 2
                    if sub == 0:
                        et2 = work.tile([P, 2, D], f8, tag="et")
                        va2 = work.tile([P, 2, 4, 2 * (Dh + 1)], f8,
                                        tag="va")
                        nc.vector.memset(va2[:, :, :, Dh:Dh + 1], 1.0)
                        nc.vector.memset(va2[:, :, :, 2 * Dh + 1:], 1.0)
                    nc.scalar.activation(out=et2[:, sub, :], in_=pk,
                                         func=Act.Exp, scale=1.0 / 64.0,
                                         bias=lnm_s[:, i:i + 1])
                    # block-diagonal per-pair va: cols 0:65 = head 2p
                    # (v/64 | 1), cols 65:130 = head 2p+1
                    pvh = pv[:].rearrange("p (a b d) -> p a b d", a=4, b=2)
                    nc.vector.tensor_scalar_mul(
                        out=va2[:, sub, :, 0:Dh], in0=pvh[:, :, 0, :],
                        scalar1=1.0 / 64.0)
                    nc.vector.tensor_scalar_mul(
                        out=va2[:, sub, :, Dh + 1:2 * Dh + 1],
                        in0=pvh[:, :, 1, :],
                        scalar1=1.0 / 64.0)
                    if sub == 1:
                        if u_pend is not None:
                            emit_u(u_pend, start=(u_cnt == 0), stop=False)
                            u_cnt += 1
                        u_pend = (et2, va2)

                if ci % (TC // 2) == TC // 2 - 1:
                    # flush the half's last U pair, then ship it
                    emit_u(u_pend, start=(u_cnt == 0), stop=True)
                    u_pend = None
                    u_cnt = 0
                    # ---- ship this half's U partials through a pair
                    # AllReduce (the first launches at sweep-1 midpoint and
                    # hides its latency under the second half) ----
                    # u_sb[p, q, f]: head 2q+(p>=64) pair-packed on the
                    # full 128 partitions -> 2 quadrant copies per u bank
                    hf = ci // (TC // 2)
                    u_sb = work.tile([P, KC, Dh + 1], bf16, tag="u_sb")
                    for ui, u in enumerate((u0, u1)):
                        nc.scalar.copy(
                            out=u_sb[0:64, 2 * ui:2 * ui + 2, :],
                            in_=u[0:64, :, 0:Dh + 1])
                        nc.scalar.copy(
                            out=u_sb[64:P, 2 * ui:2 * ui + 2, :],
                            in_=u[64:P, :, Dh + 1:2 * (Dh + 1)])
                    nc.sync.dma_start(
                        out=cc_in_t[hf][:].rearrange(
                            "(p q f) -> p q f", p=P, q=KC),
                        in_=u_sb)
                    nc.gpsimd.collective_compute(
                        "AllReduce", Alu.add, replica_groups=PAIRS,
                        ins=[cc_in_t[hf][:]], outs=[cc_out_t[hf][:]])
            ctxS1.close()

            psQ = ctxA.enter_context(tc.tile_pool(name="psQ", bufs=3, space="PSUM"))
            psS = ctxA.enter_context(tc.tile_pool(name="psS", bufs=2, space="PSUM"))
            bq_col = None
            if has_bq:
                # bq as per-partition columns [P, KC] for the Exp bias
                bq_row = const.tile([1, D], bf16)
                nc.vector.tensor_copy(out=bq_row, in_=vec_s[:, 2, :])
                pbq = psQ.tile([P, KC], f32, tag="pbq")
                for c in range(KC):
                    nc.tensor.matmul(out=pbq[:, c:c + 1],
                                     lhsT=bq_row[:, c * P:(c + 1) * P],
                                     rhs=ones_row[:, 0:1], start=True, stop=True)
                bq_col = const.tile([P, KC], f32)
                nc.scalar.copy(out=bq_col, in_=pbq)

            # ---- sweep 2 (overlaps the AllReduce): q-path ----
            # scheduling gate: the list scheduler must not interleave the
            # q sweep into sweep 1 (that delays U completion and thus the
            # AllReduce trigger by ~30us). tile_wait_until pins sweep 2's
            # earliest sim-schedule time far past sweep 1 so the static
            # per-engine order is [sweep1 | sweep2]; on HW semaphores the
            # q sweep then starts right when sweep 1 drains and overlaps
            # the collective.
            ctxA.enter_context(tc.tile_wait_until(1.0))
            for ci in range(TC):
                tsl = slice(ci * TS, (ci + 1) * TS)
                qs_ps = psS.tile([16, TS], f32, tag="qs")
                for c in range(KC):
                    qt_ps = psQ.tile([P, TS], f32, tag="qt")
                    for j in range(0, KC, 2):
                        nc.tensor.matmul(out=qt_ps,
                                         lhsT=wq_s[:, j:j + 2, c * P:(c + 1) * P],
                                         rhs=xn_s[:, ci, j:j + 2, :],
                                         perf_mode=DR,
                                         start=(j == 0), stop=(j == KC - 2))
                    if has_bq:
                        nc.scalar.activation(out=qe_s[:, c, tsl], in_=qt_ps,
                                             func=Act.Exp, scale=1.0 / 64.0,
                                             bias=bq_col[:, c:c + 1])
                    else:
                        nc.scalar.activation(out=qe_s[:, c, tsl], in_=qt_ps,
                                             func=Act.Exp, scale=1.0 / 64.0)
                    if c % 2 == 1:
                        nc.tensor.matmul(out=qs_ps,
                                         lhsT=pairones8[:, c - 1:c + 1, :],
                                         rhs=qe_s[:, c - 1:c + 1, tsl],
                                         perf_mode=DR,
                                         start=(c == 1), stop=(c == KC - 1))
                # Vector copy: the Scalar queue is sweep 2's bottleneck
                nc.vector.tensor_copy(out=qsum_sb[:, tsl], in_=qs_ps[0:8, :])

        # ================= phase B =================
        # v3 design: no ysb materialization. B1 computes raw y only to
        # derive LN2 stats (ysum via the attn2-rowsum lhsT "saug" straight
        # from qe, y2sum via squared-y fp8 DoubleRow). B2 rebuilds the
        # stylized pre-silu h1 entirely in PSUM: qe2 = qe*(64*rq*r2)
        # (DVE, [128,TS] broadcast via a tiny PE matmul), then
        # h1 = attn2.T@qe2 + 64*(nm2[t] + (C/A)[l]) accumulated by two
        # matmuls, and hs = Silu ACT with per-partition scale A/64.
        with ExitStack() as ctxB:
            embB = ctxB.enter_context(tc.tile_pool(name="embB", bufs=1))
            ypool = ctxB.enter_context(tc.tile_pool(name="ypool", bufs=1))
            workB = ctxB.enter_context(tc.tile_pool(name="workB", bufs=2))
            # psY=2: with a single y bank every y matmul stalls on the
            # Square ACT drain, resetting the PE DVFS ramp
            psY = ctxB.enter_context(tc.tile_pool(name="psY", bufs=2, space="PSUM"))
            psAcc = ctxB.enter_context(tc.tile_pool(name="psAcc", bufs=1, space="PSUM"))
            psTmp = ctxB.enter_context(tc.tile_pool(name="psTmp", bufs=2, space="PSUM"))
            psH = ctxB.enter_context(tc.tile_pool(name="psH", bufs=2, space="PSUM"))
            psO = ctxB.enter_context(tc.tile_pool(name="psO", bufs=1, space="PSUM"))

            # 1/qsum batched: rq = exp(-ln(qsum)); rq64 = 64/qsum keeps the
            # fp8 qe2/y2 tensors in e4m3's normal range
            nc.scalar.activation(out=qsum_sb, in_=qsum_sb, func=Act.Ln)
            nc.scalar.activation(out=rq_bf, in_=qsum_sb, func=Act.Exp,
                                 scale=-1.0)
            ln64_t = embB.tile([8, 1], f32)
            nc.vector.memset(ln64_t, float(np.log(64.0)))
            nc.scalar.activation(out=rq64_bf, in_=qsum_sb, func=Act.Exp,
                                 scale=-1.0, bias=ln64_t)
            rq642_bf = embB.tile([8, TH], bf16)    # (64/qsum)^2 for y2 sums
            nc.vector.tensor_mul(out=rq642_bf, in0=rq64_bf, in1=rq64_bf)

            # attn state in the pair-packed [128, 4, 65] layout (head
            # 2q+(p>=64) on row p); attn2 is the block-diagonal per-pair
            # layout [128, KC, 128]
            u_fa = embB.tile([P, KC, Dh + 1], bf16)
            u_fb = embB.tile([P, KC, Dh + 1], bf16)
            nc.sync.dma_start(
                out=u_fa, in_=cc_out_t[0][:].rearrange(
                    "(p q f) -> p q f", p=P, q=KC))
            nc.sync.dma_start(
                out=u_fb, in_=cc_out_t[1][:].rearrange(
                    "(p q f) -> p q f", p=P, q=KC))
            u_f = embB.tile([P, KC, Dh + 1], f32)
            nc.vector.tensor_add(out=u_f, in0=u_fa, in1=u_fb)
            rs = embB.tile([P, KC, 1], f32)
            nc.vector.reciprocal(out=rs, in_=u_f[:, :, Dh:Dh + 1])
            for h in range(H):
                base = 64 * (h % 2)
                nc.vector.tensor_scalar_mul(
                    out=attn2[base:base + 64, h // 2, base:base + 64],
                    in0=u_f[base:base + 64, h // 2, 0:Dh],
                    scalar1=rs[base:base + 64, h // 2, :])
            # saug[k, c, m] = rowsum_l(attn2[k, c, :]) at head m=2c+(k>=64):
            # ysum comes straight from qe (kills the ysb PSUM->SBUF copies)
            rsum = embB.tile([P, KC, 1], f32)
            nc.vector.tensor_reduce(rsum, attn2[:],
                                    mybir.AxisListType.X, Alu.add)
            # saug: fp8, 16-wide (dual-fp8 ldweights needs a 16B outer
            # stride): the ysum reduction runs in DoubleRow straight off qe
            for c in range(KC):
                nc.vector.tensor_copy(out=saug[0:64, c, 2 * c:2 * c + 1],
                                      in_=rsum[0:64, c, :])
                nc.vector.tensor_copy(
                    out=saug[64:P, c, 2 * c + 1:2 * c + 2],
                    in_=rsum[64:P, c, :])
            # A/64 as per-partition columns (Silu ACT scale), built with
            # the tiny PE transpose trick
            a_row = embB.tile([1, D], bf16)
            nc.vector.tensor_copy(out=a_row, in_=vec_s[:, 0, :])
            pa_ps = psTmp.tile([P, TS], f32, tag="tmp")
            for c in range(KC):
                nc.tensor.matmul(out=pa_ps[:, c:c + 1],
                                 lhsT=a_row[:, c * P:(c + 1) * P],
                                 rhs=ones_row[:, 0:1], start=True, stop=True)
            a_col = embB.tile([P, KC], f32)
            nc.scalar.copy(out=a_col, in_=pa_ps[:, 0:KC])

            m2_t = [ypool.tile([8, TS], bf16, tag=f"m2_{ci}",
                               name=f"m2_{ci}") for ci in range(TC)]
            var_t = [ypool.tile([8, TS], f32, tag=f"var_{ci}",
                                name=f"var_{ci}") for ci in range(TC)]
            r2_t = [ypool.tile([8, TS], bf16, tag=f"r2_{ci}",
                               name=f"r2_{ci}") for ci in range(TC)]
            rqr2_t = [ypool.tile([8, TS], bf16, tag=f"rqr2_{ci}",
                                 name=f"rqr2_{ci}") for ci in range(TC)]
            # affrhs rows 0:8 = 64*nm2 per ci (written in place by the STT
            # below, partition base 0), row 8 = 1: rhs of the K=9 affine
            affrhs = ypool.tile([9, TH], bf16)
            nc.vector.memset(affrhs, 1.0)

            # ---- B1: raw y (stats only), in two halves: each half's
            # Exp/rqr2/aff section runs while the next half's B1 matmuls
            # keep the PE busy, so B2 starts with zero PE gap ----
            HF = TC // 2
            for ci in range(TC):
                tsl = slice(ci * TS, (ci + 1) * TS)
                acc = psAcc.tile([P, TS], f32, tag="acc")
                y2sb = workB.tile([P, KC, TS], bf16, tag="y2")
                for c in range(KC):
                    y_ps = psY.tile([P, TS], f32, tag="y")
                    nc.tensor.matmul(out=y_ps, lhsT=attn2[:, c, :],
                                     rhs=qe_s[:, c, tsl],
                                     start=True, stop=True)
                    nc.scalar.activation(out=y2sb[:, c, :], in_=y_ps,
                                         func=Act.Square, scale=1.0 / 64.0)
                    # ysum DR (fp8) must land at partition 0; the bf16
                    # y2 sums go to base 64
                    if c % 2 == 1:
                        nc.tensor.matmul(out=acc[0:16, :],
                                         lhsT=saug[:, c - 1:c + 1, :],
                                         rhs=qe_s[:, c - 1:c + 1, tsl],
                                         perf_mode=DR,
                                         start=(c == 1), stop=(c == KC - 1))
                    nc.tensor.matmul(out=acc[64:72, :],
                                     lhsT=pairones8b[:, c, :],
                                     rhs=y2sb[:, c, :],
                                     start=(c == 0), stop=(c == KC - 1))
                # rq-weighted per-head sums -> all-head sums (replicated
                # across 8 partitions by the all-ones lhsT). The /64 of y2
                # cancels the 64^2 of rq64^2.
                wys = workB.tile([8, TS], bf16, tag="wys")
                nc.vector.tensor_mul(out=wys, in0=acc[0:8, :],
                                     in1=rq_bf[:, tsl])
                wy2 = workB.tile([8, TS], bf16, tag="wy2")
                nc.vector.tensor_mul(out=wy2, in0=acc[64:72, :],
                                     in1=rq642_bf[:, tsl])
                ms_ps = psTmp.tile([P, TS], f32, tag="tmp")
                nc.tensor.matmul(out=ms_ps[0:8, :], lhsT=ones8x8, rhs=wys,
                                 start=True, stop=True)
                nc.tensor.matmul(out=ms_ps[64:72, :], lhsT=ones8x8, rhs=wy2,
                                 start=True, stop=True)
                nc.scalar.activation(out=m2_t[ci], in_=ms_ps[0:8, :],
                                     func=Act.Copy, scale=1.0 / D)
                nc.scalar.activation(out=var_t[ci], in_=ms_ps[64:72, :],
                                     func=Act.Copy, scale=1.0 / D)
                msq = workB.tile([8, TS], f32, tag="msq")
                nc.vector.tensor_mul(out=msq, in0=m2_t[ci], in1=m2_t[ci])
                nc.vector.tensor_sub(out=var_t[ci], in0=var_t[ci], in1=msq)
                nc.scalar.activation(out=var_t[ci], in_=var_t[ci],
                                     func=Act.Ln, bias=eps_t[0:8, :])

                if ci % HF != HF - 1:
                    continue
                # end of a half: zero bias from its LAST Ln output forces
                # the half's Exps after its Lns (no exp<->ln ACT-table
                # ping-pong within the half)
                zb = embB.tile([8, 1], f32, tag=f"zb{ci}", name=f"zb{ci}")
                nc.vector.tensor_scalar_mul(out=zb, in0=var_t[ci][:, 0:1],
                                            scalar1=0.0)
                for cj in range(ci - HF + 1, ci + 1):
                    nc.scalar.activation(out=r2_t[cj], in_=var_t[cj],
                                         func=Act.Exp, scale=-0.5, bias=zb)
                one8z = embB.tile([8, 1], f32, tag=f"oz{ci}",
                                  name=f"oz{ci}")
                nc.vector.tensor_scalar(out=one8z, in0=r2_t[ci][:, 0:1],
                                        scalar1=0.0, scalar2=1.0,
                                        op0=Alu.mult, op1=Alu.add)
                for cj in range(ci - HF + 1, ci + 1):
                    tsj = slice(cj * TS, (cj + 1) * TS)
                    # rqr2 = r2/qsum UNSCALED: qe2 = qe*rqr2 <= r2 stays
                    # inside e4m3's 448 max even for big-r2 tokens
                    nc.vector.scalar_tensor_tensor(
                        out=rqr2_t[cj], in0=rq_bf[:, tsj],
                        scalar=one8z, in1=r2_t[cj],
                        op0=Alu.mult, op1=Alu.mult)
                    # affrhs rows 0:8 = -m2*r2 (row 8 stays all-ones)
                    nc.vector.scalar_tensor_tensor(
                        out=affrhs[0:8, tsj], in0=m2_t[cj],
                        scalar=-1.0, in1=r2_t[cj],
                        op0=Alu.mult, op1=Alu.mult)

            # ---- B2: h1 assembled in PSUM, Silu ACT, out-proj. The
            # out-proj is emitted one ci LATE so its matmuls never wait on
            # the 4-silu chain of the same ci (Tensor fills with the next
            # ci's preproc instead of stalling and dropping DVFS) ----
            def emit_outproj(ci, hs_c):
                tsl = slice(ci * TS, (ci + 1) * TS)
                for m in range(KC):
                    po = psO.tile([P, TS], f32, tag="po")
                    for c in range(KC):
                        nc.tensor.matmul(out=po,
                                         lhsT=wo_s[:, c, m * P:(m + 1) * P],
                                         rhs=hs_c[:, c, :],
                                         start=(c == 0), stop=(c == KC - 1))
                    ho = workB.tile([P, TS], bf16, tag="ho")
                    nc.vector.tensor_copy(out=ho, in_=po)
                    nc.sync.dma_start(out=h_out[m, :, tsl], in_=ho)

            op_pend = None
            for ci in range(TC):
                tsl = slice(ci * TS, (ci + 1) * TS)
                hs_c = workB.tile([P, KC, TS], bf16, tag="hs")
                for c in range(KC):
                    # [128, TS] broadcast of rqr2 rows for this chunk's pair
                    re_ps = psTmp.tile([P, TS], f32, tag="tmp")
                    nc.tensor.matmul(out=re_ps, lhsT=caff_s[0:8, c, :],
                                     rhs=rqr2_t[ci],
                                     start=True, stop=True)
                    qe2 = workB.tile([P, TS], bf16, tag="qe2")
                    nc.vector.tensor_mul(out=qe2, in0=qe_s[:, c, tsl],
                                         in1=re_ps)
                    h1_ps = psH.tile([P, TS], f32, tag="h1")
                    nc.tensor.matmul(out=h1_ps, lhsT=attn2[:, c, :],
                                     rhs=qe2, start=True, stop=False)
                    nc.tensor.matmul(out=h1_ps, lhsT=caff_s[:, c, :],
                                     rhs=affrhs[:, tsl],
                                     start=False, stop=True)
                    # hs = silu(A*(y2nd + aff)) via per-partition scale
                    nc.scalar.activation(out=hs_c[:, c, :], in_=h1_ps,
                                         func=Act.Silu,
                                         scale=a_col[:, c:c + 1])
                if op_pend is not None:
                    emit_outproj(op_pend[0], op_pend[1])
                op_pend = (ci, hs_c)
            emit_outproj(op_pend[0], op_pend[1])

    nc.compile()
    return nc


def _prep(inputs, flags):
    bf = ml_dtypes.bfloat16
    f8 = ml_dtypes.float8_e4m3fn
    x = np.asarray(inputs["x"], np.float32)
    emb = np.asarray(inputs["emb"], np.float32)
    src_mask = np.asarray(inputs["src_mask"], np.float32)
    gamma = np.asarray(inputs["gamma"], np.float32)
    beta = np.asarray(inputs["beta"], np.float32)
    gamma2 = np.asarray(inputs["gamma2"], np.float32)
    beta2 = np.asarray(inputs["beta2"], np.float32)
    emb_b = np.asarray(inputs["emb_b"], np.float32)

    # host LN1 (no gamma/beta: folded into weights)
    mu = x.mean(-1, keepdims=True)
    xc = x - mu
    var = np.mean(xc * xc, axis=-1, keepdims=True)
    xn = xc * (1.0 / np.sqrt(var + EPS))

    def foldW(Wname):
        # fp8 e4m3, pre-scaled by 64 (undone by the Exp ACT scale /
        # mask64); [P, KC, D] partition-major so the DMA runs contiguous
        W = np.asarray(inputs[Wname], np.float32)
        return np.ascontiguousarray(
            (gamma[:, None] * W * 64.0).astype(f8).reshape(KC, P, D)
            .transpose(1, 0, 2))

    wq, wk, wv = foldW("Wq"), foldW("Wk"), foldW("Wv")
    wo = np.ascontiguousarray(
        np.asarray(inputs["out_W"], np.float32).astype(bf).reshape(KC, P, D)
        .transpose(1, 0, 2))
    bq_f = np.asarray(inputs["bq"], np.float32) + beta @ np.asarray(inputs["Wq"], np.float32)
    bk_f = np.asarray(inputs["bk"], np.float32) + beta @ np.asarray(inputs["Wk"], np.float32)
    bv_f = np.asarray(inputs["bv"], np.float32) + beta @ np.asarray(inputs["Wv"], np.float32)

    # emb/stylization path fully on host
    sl_emb = emb * (1.0 / (1.0 + np.exp(-emb)))          # silu, (B, TE)
    eo = sl_emb @ np.asarray(inputs["emb_W"], np.float32) + emb_b  # (B, 2D)
    scale, shift = eo[:, :D], eo[:, D:]
    A_rows = gamma2[None, :] * (1.0 + scale)             # (B, D)
    C_rows = beta2[None, :] * (1.0 + scale) + shift      # (B, D)

    # hpair[m, c, p] = 1 when head m = 2c + (p>=64), else 0
    hpair = np.zeros((8, KC, P), np.float32)
    for c in range(KC):
        hpair[2 * c, c, 0:64] = 1.0
        hpair[2 * c + 1, c, 64:P] = 1.0

    in_maps = []
    for c in range(NCORES):
        b, th = c // 2, c % 2
        sl = slice(th * TH, (th + 1) * TH)
        # [P, TC, KC, TS]: partition-major, t-chunk-major -> per-(partition,
        # chunk) 2KB contiguous DMA runs
        xnT = np.ascontiguousarray(
            xn[b, sl].T.astype(f8).reshape(KC, P, TC, TS)
            .transpose(1, 2, 0, 3))
        # bk/bv enter PSUM before the 1/64 Exp scale -> pre-scale by 64;
        # bq is an ACT bias (applied after the scale) -> unscaled.
        # vecs row 0 = A: the Silu ACT per-partition scale.
        A_safe = np.where(np.abs(A_rows[b]) < 1e-20, 1e-20, A_rows[b])
        vecs = np.ascontiguousarray(np.stack(
            [A_rows[b], C_rows[b], bq_f, 64.0 * bv_f, 64.0 * bk_f]
        ).astype(np.float32).reshape(1, 5, D))
        # caff rows 0:8 = plain head indicator (rhs rows = -m2*r2),
        # row 8 = C/A (paired with the all-ones affrhs row)
        caff = np.concatenate(
            [hpair, (C_rows[b] / A_safe).reshape(1, KC, P)],
            axis=0).astype(bf)
        in_maps.append({
            "xn": xnT,
            # ln(mask): 0 keeps, -30 masks (exp(k/64 - 30) ~ 1e-13)
            "mask": np.ascontiguousarray(
                np.where(src_mask[b, sl, 0] > 0, 0.0, -30.0)
                .astype(np.float32)),
            "wq": wq, "wk": wk, "wv": wv, "wo": wo,
            "vecs": vecs, "hpair": np.ascontiguousarray(caff),
        })
    return in_maps


def _flags(inputs):
    gamma = np.asarray(inputs["gamma"], np.float32)
    beta = np.asarray(inputs["beta"], np.float32)

    def nz(v):
        return bool(np.any(np.asarray(v) != 0))

    bq_f = np.asarray(inputs["bq"], np.float32) + beta @ np.asarray(inputs["Wq"], np.float32)
    bk_f = np.asarray(inputs["bk"], np.float32) + beta @ np.asarray(inputs["Wk"], np.float32)
    bv_f = np.asarray(inputs["bv"], np.float32) + beta @ np.asarray(inputs["Wv"], np.float32)
    return (nz(bq_f), nz(bk_f), nz(bv_f))


def get_nc_and_inmaps(**inputs):
    flags = _flags(inputs)
    if flags not in _CACHE:
        _CACHE[flags] = _build(flags)
    return _CACHE[flags], _prep(inputs, flags)


def kernel(**inputs):
    from concourse.bass_utils import run_bass_kernel_spmd
    nc, in_maps = get_nc_and_inmaps(**inputs)
    res = run_bass_kernel_spmd(nc, in_maps, list(range(NCORES)))
    x = np.asarray(inputs["x"], np.float32)
    out_b = np.asarray(inputs["out_b"], np.float32)
    out = np.empty((B, T, D), np.float32)
    for c in range(NCORES):
        b, th = c // 2, c % 2
        sl = slice(th * TH, (th + 1) * TH)
        hT = np.asarray(res.results[c]["y"], np.float32).reshape(D, TH)
        out[b, sl] = x[b, sl] + hT.T + out_b
    return out



# revision 94
# speedup vs baseline: 1.1300x; 1.1300x over previous
"""Trainium2 Bass kernel for nn_LinearTemporalSelfAttention (B=4,T=8192,D=512,H=8).

Sharding: 8 cores = B(4) x T-halves(2). Each core owns a (b, t-half) slab
(4096 x 512) end-to-end; cross-core data is only the KV-state einsum
(sum over full T), AllReduced pair-wise.

v2 design (trace-driven rewrite of the v1 kernel):
 - Host computes LN1 ((x-mu)*rstd, exact f32; gamma/beta folded into the
   QKV weights/biases as before) and ships xn TRANSPOSED per core as
   bf16 [D, TH]. The residual x + h and the tiny emb/stylization-vector
   path (silu(emb)@emb_W) also run on host. Device input traffic halves.
 - ZERO on-device transposes (v1 spent 474us on 384 DMA_TRANSPOSEs):
   q is computed transposed (lhsT=Wq chunks stationary, rhs=xnT moving)
   and k/v in normal layout (lhsT=xnT chunks stationary, rhs=Wk/Wv) --
   both straight off the same xnT tiles. Phase B stays fully transposed
   (y.T = attn2.T @ qeT; out-proj consumes hs.T directly) and the kernel
   emits h.T; the host transposes/adds the residual.
 - No GpSimd elementwise ops (v1: 360us of Q7 software overhead), and no
   big DVE reciprocals (v1: 113us of 8cy/elem iterative divides):
   1/qsum is exp(-ln(qsum)) batched over [8, TH] on ACT; silu is
   0.5*x*(1+tanh(x/2)) with the 0.5 folded into out_W on host.
 - ACT table loads: v1 ping-ponged ln<->exp sets 125x (160us). All Ln
   usage is batched at two points (1/qsum prologue, LN2 rstd between
   B1/B2); everything else uses exp/tanh/square/copy from one set.
   ~5 loads total.
 - Per-token scalars in transposed layout (1/qsum rows, LN2 m2/rstd2,
   stylization scale/shift) are applied via tiny PE rank-1/broadcast
   matmuls into PSUM + fused DVE tensor-tensor passes.
"""
import numpy as np
import ml_dtypes

B, T, D, H, TE = 4, 8192, 512, 8, 2048
Dh = D // H          # 64
EPS = 1e-5
NCORES = 8
TH = T // 2          # 4096 rows per core
P = 128
KC = D // P          # 4 chunks of the feature dim
TS = 512             # t-columns per phase chunk
TC = TH // TS        # 8 t-chunks per core
NSUB = TS // P       # 4 row-subtiles per t-chunk
NT = TH // P         # 32 row subtiles total
CCU = 64 * H * (Dh + 1)     # 33280 floats of U_aug

_CACHE: dict = {}


def _build(flags):
    has_bq, has_bk, has_bv = flags
    from contextlib import ExitStack
    import concourse.bass as bass
    import concourse.bacc as bacc
    import concourse.tile as tile
    import concourse.mybir as mybir

    f32 = mybir.dt.float32
    bf16 = mybir.dt.bfloat16
    f8 = mybir.dt.float8e4
    DR = mybir.MatmulPerfMode.DoubleRow
    Alu = mybir.AluOpType
    Act = mybir.ActivationFunctionType

    nc = bacc.Bacc("TRN2", target_bir_lowering=False, debug=False,
                   enable_asserts=True, num_devices=NCORES)

    # host ships partition-major layouts so every DMA lands as >=2KB
    # contiguous runs per partition (512B strided runs cap at ~40% BW)
    xn_in = nc.declare_dram_parameter("xn", [P, TC, KC, TS], f8, isOutput=False)
    mk_in = nc.declare_dram_parameter("mask", [TH], f32, isOutput=False)
    wq_in = nc.declare_dram_parameter("wq", [P, KC, D], f8, isOutput=False)
    wk_in = nc.declare_dram_parameter("wk", [P, KC, D], f8, isOutput=False)
    wv_in = nc.declare_dram_parameter("wv", [P, KC, D], f8, isOutput=False)
    wo_in = nc.declare_dram_parameter("wo", [P, KC, D], bf16, isOutput=False)
    vec_in = nc.declare_dram_parameter("vecs", [1, 5, D], f32, isOutput=False)
    hp_in = nc.declare_dram_parameter("hpair", [9, KC, P], bf16, isOutput=False)
    h_out = nc.declare_dram_parameter("y", [KC, P, TH], bf16, isOutput=True)

    PAIRS = [[0, 1], [2, 3], [4, 5], [6, 7]]

    with tile.TileContext(nc) as tc, ExitStack() as ctx:
        const = ctx.enter_context(tc.tile_pool(name="const", bufs=1))
        wpool = ctx.enter_context(tc.tile_pool(name="wpool", bufs=1))
        qstash = ctx.enter_context(tc.tile_pool(name="qstash", bufs=1))
        dramp = ctx.enter_context(tc.tile_pool(name="dram", bufs=1, space="DRAM"))

        eps_t = const.tile([P, 1], f32)
        nc.vector.memset(eps_t, EPS)
        ones_row = const.tile([1, P], bf16)
        nc.vector.memset(ones_row, 1.0)
        ones_col = const.tile([P, 1], bf16)
        nc.vector.memset(ones_col, 1.0)
        # pairones8[p, c, m] = 1 if head m = 2c + (p>=64): per-chunk qsum
        # reduction lhsT with M=16 output (cols 8:16 stay zero; rows of
        # other chunks stay 0 so the PSUM accumulates all four chunks).
        # fp8 + 16-wide so the qsum / y2sum reductions run in DoubleRow
        # mode (dual-fp8 ldweights needs a 16B outer stride).
        pairones8 = const.tile([P, KC, 16], f8)
        nc.vector.memset(pairones8, 0.0)
        for c in range(KC):
            nc.vector.memset(pairones8[0:64, c, 2 * c:2 * c + 1], 1.0)
            nc.vector.memset(pairones8[64:P, c, 2 * c + 1:2 * c + 2], 1.0)
        # bf16 twin for the (bf16) y^2 sums: y^2 spans ~1e-6..10 across
        # tokens, far outside e4m3's subnormal floor, so the y2 path
        # cannot run in fp8 (underflow -> negative var -> NaN)
        pairones8b = const.tile([P, KC, 8], bf16)
        nc.vector.memset(pairones8b, 0.0)
        for c in range(KC):
            nc.vector.memset(pairones8b[0:64, c, 2 * c:2 * c + 1], 1.0)
            nc.vector.memset(pairones8b[64:P, c, 2 * c + 1:2 * c + 2], 1.0)
        # ones8x8: all-ones [8,8] -> sum-over-heads with 8x replication
        ones8x8 = const.tile([8, 8], bf16)
        nc.vector.memset(ones8x8, 1.0)
        # caff[m, c, p] = head indicator for m<8; caff[8, c, p] = 64*C/A at
        # feature c*128+p: lhsT of the K=9 affine matmul producing
        # 64*(nm2[t] + (C/A)[l]). Indicator rows first so all writable
        # rows sit at partition base 0 (HW requires 0/32/64/96 bases).
        caff_s = const.tile([9, KC, P], bf16)

        # startup-critical DMAs first: sweep 1 only needs wk/wv/mask (and
        # the first xn chunk, issued right below). wq (sweep 2), wo/vecs/
        # ahp (phase B) land during sweep 1.
        # q/k/v weights are fp8, pre-scaled by 64 on host (W std ~0.02 would
        # drown in e4m3 subnormals); the 1/64 is folded into the Exp ACT
        # scale (k, q) and the mask64 column scale (v)
        wq_s = wpool.tile([P, KC, D], f8)
        wk_s = wpool.tile([P, KC, D], f8)
        wv_s = wpool.tile([P, KC, D], f8)
        wo_s = wpool.tile([P, KC, D], bf16)
        # lnmask: 0 where kept, -30 where masked; folded into the k-path
        # Exp bias so et = exp(k)*mask needs no separate va mask multiply
        lnm_s = wpool.tile([P, NT], f32)
        vec_s = wpool.tile([1, 5, D], f32)
        # startup-critical loads in parallel across the three DMA-capable
        # queues (all rings come up ~9us into the launch preamble)
        nc.scalar.dma_start(out=wk_s, in_=wk_in[:])
        nc.gpsimd.dma_start(out=wv_s, in_=wv_in[:])

        qe_s = qstash.tile([P, KC, TH], f8)       # exp(q) transposed
        qsum_sb = qstash.tile([8, TH], f32)       # per-head q softmax sums
        rq_bf = qstash.tile([8, TH], bf16)        # 1/qsum
        rq64_bf = qstash.tile([8, TH], bf16)      # 64/qsum (fp8-scale mgmt)
        # attn2/saug allocated up front: their zero-fills run during the
        # startup DMA wait instead of on the post-collective critical path
        attn2 = qstash.tile([P, KC, P], bf16)
        nc.vector.memset(attn2, 0.0)
        saug = qstash.tile([P, KC, 16], f8)
        nc.vector.memset(saug, 0.0)

        # bf16 collective payload: halves the AllReduce bytes; U entries
        # are ~8192-token sums so the 0.4% rounding washes into ~0.5% of
        # attn2 (acceptable within the 2e-2 gate)
        cc_in_t = [dramp.tile([CCU], bf16, tag=f"cci{h}", name=f"cci{h}")
                   for h in range(2)]
        cc_out_t = [dramp.tile([CCU], bf16, tag=f"cco{h}", name=f"cco{h}")
                    for h in range(2)]

        # ================= phase A =================
        with ExitStack() as ctxA:
            xpool = ctxA.enter_context(tc.tile_pool(name="xpool", bufs=1))
            work = ctxA.enter_context(tc.tile_pool(name="work", bufs=4))
            # sweep-1 PSUM pools live only until the U copies ship; sweep 2
            # then opens its own pools (sweep-1's single-buffered pk/pv was
            # the dominant pacing bug: every pv matmul stalled ~1.3us on
            # the previous subtile's va DVE reads)
            ctxS1 = ExitStack()
            psK = ctxS1.enter_context(tc.tile_pool(name="psK", bufs=3, space="PSUM"))
            psV = ctxS1.enter_context(tc.tile_pool(name="psV", bufs=3, space="PSUM"))
            psU = ctxS1.enter_context(tc.tile_pool(name="psU", bufs=1, space="PSUM"))

            xn_s = xpool.tile([P, TC, KC, TS], f8)
            # first xn chunk next (startup critical), then the DMAs sweep 1
            # doesn't need
            nc.sync.dma_start(out=xn_s[:, 0], in_=xn_in[:, 0])
            nc.sync.dma_start(out=lnm_s,
                              in_=mk_in[:].rearrange("(n p) -> p n", p=P))
            nc.sync.dma_start(out=wq_s, in_=wq_in[:])
            nc.sync.dma_start(out=caff_s, in_=hp_in[:])
            nc.sync.dma_start(out=vec_s, in_=vec_in[:])
            nc.sync.dma_start(out=wo_s, in_=wo_in[:])

            bk_row = None
            if has_bk:
                bk_row = const.tile([1, D], bf16)
                nc.vector.tensor_copy(out=bk_row, in_=vec_s[:, 4, :])
            bv_row = None
            if has_bv:
                bv_row = const.tile([1, D], bf16)
                nc.vector.tensor_copy(out=bv_row, in_=vec_s[:, 3, :])

            # head-pair-packed U: pair p occupies [128, p%2, 130] of u0/u1;
            # quadrants [0:64, 0:65] and [64:128, 65:130] hold the two
            # heads' U_aug, the other two quadrants are ignored cross-terms
            u0 = psU.tile([P, 2, 2 * (Dh + 1)], f32, tag="u0")
            u1 = psU.tile([P, 2, 2 * (Dh + 1)], f32, tag="u1")

            # ---- sweep 1: k/v + U accumulation (feeds the AllReduce
            # as early as possible; the q sweep then runs DURING the
            # collective so the PE never idles through it). U matmuls
            # for a pair are issued one pair LATE so they never wait on
            # the et/va chain (any sub-us PE stall resets the 3us DVFS
            # ramp and halves the clock) ----
            u_pend = None
            u_cnt = 0

            def emit_u(pend, start, stop):
                pet2, pva2 = pend
                for p in range(4):
                    u = u0 if p < 2 else u1
                    nc.tensor.matmul(
                        out=u[:, p % 2, :],
                        lhsT=pet2[:, :, p * P:(p + 1) * P],
                        rhs=pva2[:, :, p, :],
                        perf_mode=DR,
                        start=(start and p % 2 == 0),
                        stop=(stop and p % 2 == 1))

            for ci in range(TC):
                if ci > 0:
                    nc.sync.dma_start(out=xn_s[:, ci], in_=xn_in[:, ci])
                for ti in range(NSUB):
                    i = ci * NSUB + ti
                    ssl = slice(ti * P, (ti + 1) * P)
                    pk = psK.tile([P, D], f32, tag="pk")
                    pv = psV.tile([P, D], f32, tag="pv")
                    for j in range(0, KC, 2):
                        nc.tensor.matmul(out=pk,
                                         lhsT=xn_s[:, ci, j:j + 2, ssl],
                                         rhs=wk_s[:, j:j + 2, :],
                                         perf_mode=DR,
                                         start=(j == 0),
                                         stop=(j == KC - 2 and not has_bk))
                        nc.tensor.matmul(out=pv,
                                         lhsT=xn_s[:, ci, j:j + 2, ssl],
                                         rhs=wv_s[:, j:j + 2, :],
                                         perf_mode=DR,
                                         start=(j == 0),
                                         stop=(j == KC - 2 and not has_bv))
                    if has_bk:
                        nc.tensor.matmul(out=pk, lhsT=ones_row, rhs=bk_row,
                                         start=False, stop=True)
                    if has_bv:
                        nc.tensor.matmul(out=pv, lhsT=ones_row, rhs=bv_row,
                                         start=False, stop=True)
                    # et = exp(k)*mask via the ln(mask) ACT bias: the va
                    # mask multiplies disappear (ones-cols memset once per
                    # pair). et/va fp8, paired across two consecutive
                    # subtiles: U accumulation runs in DoubleRow (U
                    # averages over 8192 tokens, e4m3 noise washes out).
                    sub = i % 2
                    if sub == 0:
                        et2 = work.tile([P, 2, D], f8, tag="et")
                        va2 = work.tile([P, 2, 4, 2 * (Dh + 1)], f8,
                                        tag="va")
                        nc.vector.memset(va2[:, :, :, Dh:Dh + 1], 1.0)
                        nc.vector.memset(va2[:, :, :, 2 * Dh + 1:], 1.0)
                    nc.scalar.activation(out=et2[:, sub, :], in_=pk,
                                         func=Act.Exp, scale=1.0 / 64.0,
                                         bias=lnm_s[:, i:i + 1])
                    # block-diagonal per-pair va: cols 0:65 = head 2p
                    # (v/64 | 1), cols 65:130 = head 2p+1
                    pvh = pv[:].rearrange("p (a b d) -> p a b d", a=4, b=2)
                    nc.vector.tensor_scalar_mul(
                        out=va2[:, sub, :, 0:Dh], in0=pvh[:, :, 0, :],
                        scalar1=1.0 / 64.0)
                    nc.vector.tensor_scalar_mul(
                        out=va2[:, sub, :, Dh + 1:2 * Dh + 1],
                        in0=pvh[:, :, 1, :],
                        scalar1=1.0 / 64.0)
                    if sub == 1:
                        if u_pend is not None:
                            emit_u(u_pend, start=(u_cnt == 0), stop=False)
                            u_cnt += 1
                        u_pend = (et2, va2)

                if ci % (TC // 2) == TC // 2 - 1:
                    # flush the half's last U pair, then ship it
                    emit_u(u_pend, start=(u_cnt == 0), stop=True)
                    u_pend = None
                    u_cnt = 0
                    # ---- ship this half's U partials through a pair
                    # AllReduce (the first launches at sweep-1 midpoint and
                    # hides its latency under the second half) ----
                    # u_sb[p, q, f]: head 2q+(p>=64) pair-packed on the
                    # full 128 partitions -> 2 quadrant copies per u bank
                    hf = ci // (TC // 2)
                    u_sb = work.tile([P, KC, Dh + 1], bf16, tag="u_sb")
                    for ui, u in enumerate((u0, u1)):
                        nc.scalar.copy(
                            out=u_sb[0:64, 2 * ui:2 * ui + 2, :],
                            in_=u[0:64, :, 0:Dh + 1])
                        nc.scalar.copy(
                            out=u_sb[64:P, 2 * ui:2 * ui + 2, :],
                            in_=u[64:P, :, Dh + 1:2 * (Dh + 1)])
                    nc.sync.dma_start(
                        out=cc_in_t[hf][:].rearrange(
                            "(p q f) -> p q f", p=P, q=KC),
                        in_=u_sb)
                    nc.gpsimd.collective_compute(
                        "AllReduce", Alu.add, replica_groups=PAIRS,
                        ins=[cc_in_t[hf][:]], outs=[cc_out_t[hf][:]])
            ctxS1.close()

            psQ = ctxA.enter_context(tc.tile_pool(name="psQ", bufs=3, space="PSUM"))
            psS = ctxA.enter_context(tc.tile_pool(name="psS", bufs=2, space="PSUM"))
            bq_col = None
            if has_bq:
                # bq as per-partition columns [P, KC] for the Exp bias
                bq_row = const.tile([1, D], bf16)
                nc.vector.tensor_copy(out=bq_row, in_=vec_s[:, 2, :])
                pbq = psQ.tile([P, KC], f32, tag="pbq")
                for c in range(KC):
                    nc.tensor.matmul(out=pbq[:, c:c + 1],
                                     lhsT=bq_row[:, c * P:(c + 1) * P],
                                     rhs=ones_row[:, 0:1], start=True, stop=True)
                bq_col = const.tile([P, KC], f32)
                nc.scalar.copy(out=bq_col, in_=pbq)

            # ---- sweep 2 (overlaps the AllReduce): q-path ----
            # scheduling gate: the list scheduler must not interleave the
            # q sweep into sweep 1 (that delays U completion and thus the
            # AllReduce trigger by ~30us). tile_wait_until pins sweep 2's
            # earliest sim-schedule time far past sweep 1 so the static
            # per-engine order is [sweep1 | sweep2]; on HW semaphores the
            # q sweep then starts right when sweep 1 drains and overlaps
            # the collective.
            ctxA.enter_context(tc.tile_wait_until(1.0))
            for ci in range(TC):
                tsl = slice(ci * TS, (ci + 1) * TS)
                qs_ps = psS.tile([16, TS], f32, tag="qs")
                for c in range(KC):
                    qt_ps = psQ.tile([P, TS], f32, tag="qt")
                    for j in range(0, KC, 2):
                        nc.tensor.matmul(out=qt_ps,
                                         lhsT=wq_s[:, j:j + 2, c * P:(c + 1) * P],
                                         rhs=xn_s[:, ci, j:j + 2, :],
                                         perf_mode=DR,
                                         start=(j == 0), stop=(j == KC - 2))
                    if has_bq:
                        nc.scalar.activation(out=qe_s[:, c, tsl], in_=qt_ps,
                                             func=Act.Exp, scale=1.0 / 64.0,
                                             bias=bq_col[:, c:c + 1])
                    else:
                        nc.scalar.activation(out=qe_s[:, c, tsl], in_=qt_ps,
                                             func=Act.Exp, scale=1.0 / 64.0)
                    if c % 2 == 1:
                        nc.tensor.matmul(out=qs_ps,
                                         lhsT=pairones8[:, c - 1:c + 1, :],
                                         rhs=qe_s[:, c - 1:c + 1, tsl],
                                         perf_mode=DR,
                                         start=(c == 1), stop=(c == KC - 1))
                # Vector copy: the Scalar queue is sweep 2's bottleneck
                nc.vector.tensor_copy(out=qsum_sb[:, tsl], in_=qs_ps[0:8, :])

        # ================= phase B =================
        # v3 design: no ysb materialization. B1 computes raw y only to
        # derive LN2 stats (ysum via the attn2-rowsum lhsT "saug" straight
        # from qe, y2sum via squared-y fp8 DoubleRow). B2 rebuilds the
        # stylized pre-silu h1 entirely in PSUM: qe2 = qe*(64*rq*r2)
        # (DVE, [128,TS] broadcast via a tiny PE matmul), then
        # h1 = attn2.T@qe2 + 64*(nm2[t] + (C/A)[l]) accumulated by two
        # matmuls, and hs = Silu ACT with per-partition scale A/64.
        with ExitStack() as ctxB:
            embB = ctxB.enter_context(tc.tile_pool(name="embB", bufs=1))
            ypool = ctxB.enter_context(tc.tile_pool(name="ypool", bufs=1))
            workB = ctxB.enter_context(tc.tile_pool(name="workB", bufs=2))
            # psY=2: with a single y bank every y matmul stalls on the
            # Square ACT drain, resetting the PE DVFS ramp
            psY = ctxB.enter_context(tc.tile_pool(name="psY", bufs=2, space="PSUM"))
            psAcc = ctxB.enter_context(tc.tile_pool(name="psAcc", bufs=1, space="PSUM"))
            psTmp = ctxB.enter_context(tc.tile_pool(name="psTmp", bufs=2, space="PSUM"))
            psH = ctxB.enter_context(tc.tile_pool(name="psH", bufs=2, space="PSUM"))
            psO = ctxB.enter_context(tc.tile_pool(name="psO", bufs=1, space="PSUM"))

            # 1/qsum batched: rq = exp(-ln(qsum)); rq64 = 64/qsum keeps the
            # fp8 qe2/y2 tensors in e4m3's normal range
            nc.scalar.activation(out=qsum_sb, in_=qsum_sb, func=Act.Ln)
            nc.scalar.activation(out=rq_bf, in_=qsum_sb, func=Act.Exp,
                                 scale=-1.0)
            ln64_t = embB.tile([8, 1], f32)
            nc.vector.memset(ln64_t, float(np.log(64.0)))
            nc.scalar.activation(out=rq64_bf, in_=qsum_sb, func=Act.Exp,
                                 scale=-1.0, bias=ln64_t)
            rq642_bf = embB.tile([8, TH], bf16)    # (64/qsum)^2 for y2 sums
            nc.vector.tensor_mul(out=rq642_bf, in0=rq64_bf, in1=rq64_bf)

            # attn state in the pair-packed [128, 4, 65] layout (head
            # 2q+(p>=64) on row p); attn2 is the block-diagonal per-pair
            # layout [128, KC, 128]
            u_fa = embB.tile([P, KC, Dh + 1], bf16)
            u_fb = embB.tile([P, KC, Dh + 1], bf16)
            nc.sync.dma_start(
                out=u_fa, in_=cc_out_t[0][:].rearrange(
                    "(p q f) -> p q f", p=P, q=KC))
            nc.sync.dma_start(
                out=u_fb, in_=cc_out_t[1][:].rearrange(
                    "(p q f) -> p q f", p=P, q=KC))
            u_f = embB.tile([P, KC, Dh + 1], f32)
            nc.vector.tensor_add(out=u_f, in0=u_fa, in1=u_fb)
            rs = embB.tile([P, KC, 1], f32)
            nc.vector.reciprocal(out=rs, in_=u_f[:, :, Dh:Dh + 1])
            for h in range(H):
                base = 64 * (h % 2)
                nc.vector.tensor_scalar_mul(
                    out=attn2[base:base + 64, h // 2, base:base + 64],
                    in0=u_f[base:base + 64, h // 2, 0:Dh],
                    scalar1=rs[base:base + 64, h // 2, :])
            # saug[k, c, m] = rowsum_l(attn2[k, c, :]) at head m=2c+(k>=64):
            # ysum comes straight from qe (kills the ysb PSUM->SBUF copies)
            rsum = embB.tile([P, KC, 1], f32)
            nc.vector.tensor_reduce(rsum, attn2[:],
                                    mybir.AxisListType.X, Alu.add)
            # saug: fp8, 16-wide (dual-fp8 ldweights needs a 16B outer
            # stride): the ysum reduction runs in DoubleRow straight off qe
            for c in range(KC):
                nc.vector.tensor_copy(out=saug[0:64, c, 2 * c:2 * c + 1],
                                      in_=rsum[0:64, c, :])
                nc.vector.tensor_copy(
                    out=saug[64:P, c, 2 * c + 1:2 * c + 2],
                    in_=rsum[64:P, c, :])
            # A/64 as per-partition columns (Silu ACT scale), built with
            # the tiny PE transpose trick
            a_row = embB.tile([1, D], bf16)
            nc.vector.tensor_copy(out=a_row, in_=vec_s[:, 0, :])
            pa_ps = psTmp.tile([P, TS], f32, tag="tmp")
            for c in range(KC):
                nc.tensor.matmul(out=pa_ps[:, c:c + 1],
                                 lhsT=a_row[:, c * P:(c + 1) * P],
                                 rhs=ones_row[:, 0:1], start=True, stop=True)
            a_col = embB.tile([P, KC], f32)
            nc.scalar.copy(out=a_col, in_=pa_ps[:, 0:KC])

            m2_t = [ypool.tile([8, TS], bf16, tag=f"m2_{ci}",
                               name=f"m2_{ci}") for ci in range(TC)]
            var_t = [ypool.tile([8, TS], f32, tag=f"var_{ci}",
                                name=f"var_{ci}") for ci in range(TC)]
            r2_t = [ypool.tile([8, TS], bf16, tag=f"r2_{ci}",
                               name=f"r2_{ci}") for ci in range(TC)]
            rqr2_t = [ypool.tile([8, TS], bf16, tag=f"rqr2_{ci}",
                                 name=f"rqr2_{ci}") for ci in range(TC)]
            # affrhs rows 0:8 = 64*nm2 per ci (written in place by the STT
            # below, partition base 0), row 8 = 1: rhs of the K=9 affine
            affrhs = ypool.tile([9, TH], bf16)
            nc.vector.memset(affrhs, 1.0)

            # ---- B1: raw y (stats only), in two halves: each half's
            # Exp/rqr2/aff section runs while the next half's B1 matmuls
            # keep the PE busy, so B2 starts with zero PE gap ----
            HF = TC // 2
            for ci in range(TC):
                tsl = slice(ci * TS, (ci + 1) * TS)
                acc = psAcc.tile([P, TS], f32, tag="acc")
                y2sb = workB.tile([P, KC, TS], bf16, tag="y2")
                for c in range(KC):
                    y_ps = psY.tile([P, TS], f32, tag="y")
                    nc.tensor.matmul(out=y_ps, lhsT=attn2[:, c, :],
                                     rhs=qe_s[:, c, tsl],
                                     start=True, stop=True)
                    nc.scalar.activation(out=y2sb[:, c, :], in_=y_ps,
                                         func=Act.Square, scale=1.0 / 64.0)
                    # ysum DR (fp8) must land at partition 0; the bf16
                    # y2 sums go to base 64
                    if c % 2 == 1:
                        nc.tensor.matmul(out=acc[0:16, :],
                                         lhsT=saug[:, c - 1:c + 1, :],
                                         rhs=qe_s[:, c - 1:c + 1, tsl],
                                         perf_mode=DR,
                                         start=(c == 1), stop=(c == KC - 1))
                    nc.tensor.matmul(out=acc[64:72, :],
                                     lhsT=pairones8b[:, c, :],
                                     rhs=y2sb[:, c, :],
                                     start=(c == 0), stop=(c == KC - 1))
                # rq-weighted per-head sums -> all-head sums (replicated
                # across 8 partitions by the all-ones lhsT). The /64 of y2
                # cancels the 64^2 of rq64^2.
                wys = workB.tile([8, TS], bf16, tag="wys")
                nc.vector.tensor_mul(out=wys, in0=acc[0:8, :],
                                     in1=rq_bf[:, tsl])
                wy2 = workB.tile([8, TS], bf16, tag="wy2")
                nc.vector.tensor_mul(out=wy2, in0=acc[64:72, :],
                                     in1=rq642_bf[:, tsl])
                ms_ps = psTmp.tile([P, TS], f32, tag="tmp")
                nc.tensor.matmul(out=ms_ps[0:8, :], lhsT=ones8x8, rhs=wys,
                                 start=True, stop=True)
                nc.tensor.matmul(out=ms_ps[64:72, :], lhsT=ones8x8, rhs=wy2,
                                 start=True, stop=True)
                nc.scalar.activation(out=m2_t[ci], in_=ms_ps[0:8, :],
                                     func=Act.Copy, scale=1.0 / D)
                nc.scalar.activation(out=var_t[ci], in_=ms_ps[64:72, :],
                                     func=Act.Copy, scale=1.0 / D)
                msq = workB.tile([8, TS], f32, tag="msq")
                nc.vector.tensor_mul(out=msq, in0=m2_t[ci], in1=m2_t[ci])
                nc.vector.tensor_sub(out=var_t[ci], in0=var_t[ci], in1=msq)
                nc.scalar.activation(out=var_t[ci], in_=var_t[ci],
                                     func=Act.Ln, bias=eps_t[0:8, :])

                if ci % HF != HF - 1:
                    continue
                # end of a half: zero bias from its LAST Ln output forces
                # the half's Exps after its Lns (no exp<->ln ACT-table
                # ping-pong within the half)
                zb = embB.tile([8, 1], f32, tag=f"zb{ci}", name=f"zb{ci}")
                nc.vector.tensor_scalar_mul(out=zb, in0=var_t[ci][:, 0:1],
                                            scalar1=0.0)
                for cj in range(ci - HF + 1, ci + 1):
                    nc.scalar.activation(out=r2_t[cj], in_=var_t[cj],
                                         func=Act.Exp, scale=-0.5, bias=zb)
                one8z = embB.tile([8, 1], f32, tag=f"oz{ci}",
                                  name=f"oz{ci}")
                nc.vector.tensor_scalar(out=one8z, in0=r2_t[ci][:, 0:1],
                                        scalar1=0.0, scalar2=1.0,
                                        op0=Alu.mult, op1=Alu.add)
                for cj in range(ci - HF + 1, ci + 1):
                    tsj = slice(cj * TS, (cj + 1) * TS)
                    # rqr2 = r2/qsum UNSCALED: qe2 = qe*rqr2 <= r2 stays
                    # inside e4m3's 448 max even for big-r2 tokens
                    nc.vector.scalar_tensor_tensor(
                        out=rqr2_t[cj], in0=rq_bf[:, tsj],
                        scalar=one8z, in1=r2_t[cj],
                        op0=Alu.mult, op1=Alu.mult)
                    # affrhs rows 0:8 = -m2*r2 (row 8 stays all-ones)
                    nc.vector.scalar_tensor_tensor(
                        out=affrhs[0:8, tsj], in0=m2_t[cj],
                        scalar=-1.0, in1=r2_t[cj],
                        op0=Alu.mult, op1=Alu.mult)

            # ---- B2: h1 assembled in PSUM, Silu ACT, out-proj. The
            # out-proj is emitted one ci LATE so its matmuls never wait on
            # the 4-silu chain of the same ci (Tensor fills with the next
            # ci's preproc instead of stalling and dropping DVFS) ----
            def emit_outproj(ci, hs_c):
                tsl = slice(ci * TS, (ci + 1) * TS)
                for m in range(KC):
                    po = psO.tile([P, TS], f32, tag="po")
                    for c in range(KC):
                        nc.tensor.matmul(out=po,
                                         lhsT=wo_s[:, c, m * P:(m + 1) * P],
                                         rhs=hs_c[:, c, :],
                                         start=(c == 0), stop=(c == KC - 1))
                    ho = workB.tile([P, TS], bf16, tag="ho")
                    nc.vector.tensor_copy(out=ho, in_=po)
                    nc.sync.dma_start(out=h_out[m, :, tsl], in_=ho)

            for ci in range(TC):
                tsl = slice(ci * TS, (ci + 1) * TS)
                hs_c = workB.tile([P, KC, TS], bf16, tag="hs")
                for c in range(KC):
                    # [128, TS] broadcast of rqr2 rows for this chunk's pair
                    re_ps = psTmp.tile([P, TS], f32, tag="tmp")
                    nc.tensor.matmul(out=re_ps, lhsT=caff_s[0:8, c, :],
                                     rhs=rqr2_t[ci],
                                     start=True, stop=True)
                    qe2 = workB.tile([P, TS], bf16, tag="qe2")
                    nc.vector.tensor_mul(out=qe2, in0=qe_s[:, c, tsl],
                                         in1=re_ps)
                    h1_ps = psH.tile([P, TS], f32, tag="h1")
                    nc.tensor.matmul(out=h1_ps, lhsT=attn2[:, c, :],
                                     rhs=qe2, start=True, stop=False)
                    nc.tensor.matmul(out=h1_ps, lhsT=caff_s[:, c, :],
                                     rhs=affrhs[:, tsl],
                                     start=False, stop=True)
                    # hs = silu(A*(y2nd + aff)) via per-partition scale
                    nc.scalar.activation(out=hs_c[:, c, :], in_=h1_ps,
                                         func=Act.Silu,
                                         scale=a_col[:, c:c + 1])
                emit_outproj(ci, hs_c)

    nc.compile()
    return nc


def _prep(inputs, flags):
    bf = ml_dtypes.bfloat16
    f8 = ml_dtypes.float8_e4m3fn
    x = np.asarray(inputs["x"], np.float32)
    emb = np.asarray(inputs["emb"], np.float32)
    src_mask = np.asarray(inputs["src_mask"], np.float32)
    gamma = np.asarray(inputs["gamma"], np.float32)
    beta = np.asarray(inputs["beta"], np.float32)
    gamma2 = np.asarray(inputs["gamma2"], np.float32)
    beta2 = np.asarray(inputs["beta2"], np.float32)
    emb_b = np.asarray(inputs["emb_b"], np.float32)

    # host LN1 (no gamma/beta: folded into weights)
    mu = x.mean(-1, keepdims=True)
    xc = x - mu
    var = np.mean(xc * xc, axis=-1, keepdims=True)
    xn = xc * (1.0 / np.sqrt(var + EPS))

    def foldW(Wname):
        # fp8 e4m3, pre-scaled by 64 (undone by the Exp ACT scale /
        # mask64); [P, KC, D] partition-major so the DMA runs contiguous
        W = np.asarray(inputs[Wname], np.float32)
        return np.ascontiguousarray(
            (gamma[:, None] * W * 64.0).astype(f8).reshape(KC, P, D)
            .transpose(1, 0, 2))

    wq, wk, wv = foldW("Wq"), foldW("Wk"), foldW("Wv")
    wo = np.ascontiguousarray(
        np.asarray(inputs["out_W"], np.float32).astype(bf).reshape(KC, P, D)
        .transpose(1, 0, 2))
    bq_f = np.asarray(inputs["bq"], np.float32) + beta @ np.asarray(inputs["Wq"], np.float32)
    bk_f = np.asarray(inputs["bk"], np.float32) + beta @ np.asarray(inputs["Wk"], np.float32)
    bv_f = np.asarray(inputs["bv"], np.float32) + beta @ np.asarray(inputs["Wv"], np.float32)

    # emb/stylization path fully on host
    sl_emb = emb * (1.0 / (1.0 + np.exp(-emb)))          # silu, (B, TE)
    eo = sl_emb @ np.asarray(inputs["emb_W"], np.float32) + emb_b  # (B, 2D)
    scale, shift = eo[:, :D], eo[:, D:]
    A_rows = gamma2[None, :] * (1.0 + scale)             # (B, D)
    C_rows = beta2[None, :] * (1.0 + scale) + shift      # (B, D)

    # hpair[m, c, p] = 1 when head m = 2c + (p>=64), else 0
    hpair = np.zeros((8, KC, P), np.float32)
    for c in range(KC):
        hpair[2 * c, c, 0:64] = 1.0
        hpair[2 * c + 1, c, 64:P] = 1.0

    in_maps = []
    for c in range(NCORES):
        b, th = c // 2, c % 2
        sl = slice(th * TH, (th + 1) * TH)
        # [P, TC, KC, TS]: partition-major, t-chunk-major -> per-(partition,
        # chunk) 2KB contiguous DMA runs
        xnT = np.ascontiguousarray(
            xn[b, sl].T.astype(f8).reshape(KC, P, TC, TS)
            .transpose(1, 2, 0, 3))
        # bk/bv enter PSUM before the 1/64 Exp scale -> pre-scale by 64;
        # bq is an ACT bias (applied after the scale) -> unscaled.
        # vecs row 0 = A: the Silu ACT per-partition scale.
        A_safe = np.where(np.abs(A_rows[b]) < 1e-20, 1e-20, A_rows[b])
        vecs = np.ascontiguousarray(np.stack(
            [A_rows[b], C_rows[b], bq_f, 64.0 * bv_f, 64.0 * bk_f]
        ).astype(np.float32).reshape(1, 5, D))
        # caff rows 0:8 = plain head indicator (rhs rows = -m2*r2),
        # row 8 = C/A (paired with the all-ones affrhs row)
        caff = np.concatenate(
            [hpair, (C_rows[b] / A_safe).reshape(1, KC, P)],
            axis=0).astype(bf)
        in_maps.append({
            "xn": xnT,
            # ln(mask): 0 keeps, -30 masks (exp(k/64 - 30) ~ 1e-13)
            "mask": np.ascontiguousarray(
                np.where(src_mask[b, sl, 0] > 0, 0.0, -30.0)
                .astype(np.float32)),
            "wq": wq, "wk": wk, "wv": wv, "wo": wo,
            "vecs": vecs, "hpair": np.ascontiguousarray(caff),
        })
    return in_maps


def _flags(inputs):
    gamma = np.asarray(inputs["gamma"], np.float32)
    beta = np.asarray(inputs["beta"], np.float32)

    def nz(v):
        return bool(np.any(np.asarray(v) != 0))

    bq_f = np.asarray(inputs["bq"], np.float32) + beta @ np.asarray(inputs["Wq"], np.float32)
    bk_f = np.asarray(inputs["bk"], np.float32) + beta @ np.asarray(inputs["Wk"], np.float32)
    bv_f = np.asarray(inputs["bv"], np.float32) + beta @ np.asarray(inputs["Wv"], np.float32)
    return (nz(bq_f), nz(bk_f), nz(bv_f))


def get_nc_and_inmaps(**inputs):
    flags = _flags(inputs)
    if flags not in _CACHE:
        _CACHE[flags] = _build(flags)
    return _CACHE[flags], _prep(inputs, flags)


def kernel(**inputs):
    from concourse.bass_utils import run_bass_kernel_spmd
    nc, in_maps = get_nc_and_inmaps(**inputs)
    res = run_bass_kernel_spmd(nc, in_maps, list(range(NCORES)))
    x = np.asarray(inputs["x"], np.float32)
    out_b = np.asarray(inputs["out_b"], np.float32)
    out = np.empty((B, T, D), np.float32)
    for c in range(NCORES):
        b, th = c // 2, c % 2
        sl = slice(th * TH, (th + 1) * TH)
        hT = np.asarray(res.results[c]["y"], np.float32).reshape(D, TH)
        out[b, sl] = x[b, sl] + hT.T + out_b
    return out

